# revision 23
# baseline (speedup 1.0000x reference)
"""Trainium2 Bass kernel: segmented statistical moments (mean/var/skew/kurt).

Strategy (8 NeuronCores, one SPMD program):
  - 4096 sorted segments -> 512 consecutive segments per core, grouped into
    16 windows of W=32 segments. Host re-packs nodes serpentine-style:
    within a window, slot (p, t) holds node p*t_win + t of the window's
    segment stream; the per-slot one-hot (vs the window's 32 segments) is
    precomputed on the host in bf16 and shipped with x in one DMA per chunk.
  - All node data flows in bf16. Per chunk (G=16 tiles), ACT computes x^2
    (Square) plus a small slice of x^4; DVE computes x^3 = x*x2 and the
    rest of x^4 = x2*x2 (2x-packed bf16). GpSimd is left idle on purpose:
    its SBUF traffic poisons concurrent DVE tensor_tensor throughput.
  - Per 128-node tile: one matmul onehot.T @ [x | x^2 | x^3 | x^4] (bf16,
    free=256, 1 cycle/row; the per-matmul LDWEIGHTS ~96ns hides under the
    ~107ns compute) accumulates per-segment power sums in f32 PSUM.
    start=True on each window's first tile resets that window's 256-col
    half-bank (PSUM reset granularity is 256 columns; never interleave two
    accumulation chains inside one 256-col block).
  - Finished windows are copied PSUM->SBUF on ACT (keeps DVE clean) and
    DMA'd out. Host finalizes moments (float64) and concatenates cores.
"""

import sys

if "/opt/trn_rl_repo" not in sys.path:
    sys.path.insert(0, "/opt/trn_rl_repo")

import numpy as np
import ml_dtypes

BF16 = ml_dtypes.bfloat16

N_CORES = 8
B = 4096
C = 64
SEGS_PER_CORE = B // N_CORES      # 512
W = 32                            # segments per window
WINDOWS_PER_CORE = SEGS_PER_CORE // W   # 16
G = 16                            # 128-node tiles per chunk
GC = G * C                        # 1024
OH = G * W                        # 512 one-hot elems per partition per chunk
OH_PAD = GC                       # one-hot region padded: oh at [OH:2*OH)
ACT_X4_G = 3                      # g-slices of x^4 on ACT (rest on DVE)

_prog_cache = {}
TRACE = False


def _postprocess(nc, mybir, max_waits=1):
    """Walrus allows only one sync-wait per instruction; move extras onto
    standalone EventSemaphore instructions. Also flag one-hot matmul
    weights as {0,1}."""
    n = [0]

    def mk(engine, waits):
        wi = mybir.InstEventSemaphore(name=f"xw_{n[0]}", ins=[], outs=[])
        n[0] += 1
        wi.engine = engine
        wi.sync_info = mybir.SyncInfo(on_wait=list(waits), on_update=[])
        return wi

    for bb in nc.main_func.blocks:
        out = []
        for ins in bb.instructions:
            if ins.opcode == "Matmult":
                ins.is_weight_onezero = True
            si = ins.sync_info
            if (
                si is not None
                and len(si.on_wait) > max_waits
                and ins.opcode != "EventSemaphore"
            ):
                waits = list(si.on_wait)
                for w in waits[:-max_waits]:
                    out.append(mk(ins.engine, [w]))
                ins.sync_info = mybir.SyncInfo(
                    on_wait=waits[-max_waits:], on_update=list(si.on_update)
                )
            out.append(ins)
        bb.instructions = out


def _build_program(t_win):
    import concourse.bass as bass
    import concourse.tile as tile
    import concourse.mybir as mybir

    F32 = mybir.dt.float32
    BF = mybir.dt.bfloat16
    TT = mybir.AluOpType

    tiles = WINDOWS_PER_CORE * t_win
    assert tiles % G == 0
    chunks = tiles // G
    n_banks = WINDOWS_PER_CORE // 2   # two windows per PSUM bank

    nc = bass.Bass()
    # per chunk row: [oh (512)| x (1024)] bf16, landing at cmb[:, OH:OH+1536)
    xoh_d = nc.dram_tensor(
        "xoh", [chunks, 128, OH + GC], BF, kind="ExternalInput"
    )
    out_d = nc.dram_tensor("out", [SEGS_PER_CORE, 4 * C], F32, kind="ExternalOutput")

    with tile.TileContext(nc) as tc:
        with (
            tc.tile_pool(name="const", bufs=1) as const,
            tc.tile_pool(name="cmbp", bufs=8) as cmbp,
            tc.tile_pool(name="psp", bufs=1, space="PSUM") as psp,
            tc.tile_pool(name="outp", bufs=4) as outp,
        ):
            banks = [
                psp.tile([128, 512], F32, name=f"bank{k}", tag=f"bank{k}")
                for k in range(n_banks)
            ]

            # PE p-state warmup: ~50 dependency-free matmuls on zeroed SBUF
            # ramp the tensor engine to 2.4 GHz during the pipeline-fill dead
            # time. Results land in bank 0, which the first real window
            # matmul resets via start=True.
            warm = const.tile([128, 256], BF)
            nc.vector.memset(warm[:].bitcast(mybir.dt.uint16), 0)
            for i in range(12):
                nc.tensor.matmul(
                    banks[0][0:W, 0:256],
                    warm[:, 0:W],
                    warm[:, :],
                    start=(i == 0),
                    stop=(i == 11),
                    skip_group_check=True,
                )

            for k in range(chunks):
                cmb = cmbp.tile([128, OH_PAD + 4 * GC], BF)
                nc.sync.dma_start(
                    out=cmb[:, OH : OH + OH + GC], in_=xoh_d[k]
                )
                oh_t = cmb[:, OH : 2 * OH].rearrange("p (g w) -> p g w", w=W)
                o = OH_PAD
                xr = cmb[:, o : o + GC]
                x2r = cmb[:, o + GC : o + 2 * GC]
                x3r = cmb[:, o + 2 * GC : o + 3 * GC]
                x4r = cmb[:, o + 3 * GC : o + 4 * GC]

                nc.scalar.activation(
                    out=x2r, in_=xr, func=mybir.ActivationFunctionType.Square
                )
                nc.vector.tensor_tensor(out=x3r, in0=xr, in1=x2r, op=TT.mult)
                sa = ACT_X4_G * C
                if sa:
                    nc.scalar.activation(
                        out=x4r[:, 0:sa],
                        in_=x2r[:, 0:sa],
                        func=mybir.ActivationFunctionType.Square,
                    )
                nc.vector.tensor_tensor(
                    out=x4r[:, sa:], in0=x2r[:, sa:], in1=x2r[:, sa:], op=TT.mult
                )

                pow4 = cmb[:, o : o + 4 * GC].rearrange("p (s gc) -> p s gc", s=4)
                for g in range(G):
                    t = k * G + g
                    w = t // t_win
                    bank = banks[w // 2]
                    col0 = (w % 2) * 256
                    nc.tensor.matmul(
                        bank[0:W, col0 : col0 + 256],
                        oh_t[:, g, :],
                        pow4[:, :, g * C : (g + 1) * C],
                        start=t % t_win == 0,
                        stop=(t + 1) % t_win == 0,
                        skip_group_check=True,
                    )
                    # copy a bank only after BOTH its windows are done, so no
                    # copy ever false-shares the bank tile with a start-matmul
                    if (t + 1) % (2 * t_win) == 0:
                        kb = w // 2
                        o_t = outp.tile([W, 512], F32, name=f"o{kb % 4}", tag="o")
                        nc.scalar.copy(o_t[:, :], bank[0:W, :])
                        row0 = kb * 2 * W
                        od = out_d[:]
                        out_ap = bass.AP(
                            tensor=od.tensor,
                            offset=od.offset + row0 * 256,
                            ap=[[256, W], [W * 256, 2], [1, 256]],
                        )
                        in_ap = o_t[:].rearrange("p (j c) -> p j c", j=2)
                        nc.sync.dma_start(out=out_ap, in_=in_ap)

    _postprocess(nc, mybir)
    return nc


def _prepare_inputs(graph, batch_indices):
    idx = np.asarray(batch_indices).astype(np.int64)
    x = np.ascontiguousarray(np.asarray(graph, dtype=np.float32))
    n = idx.shape[0]

    counts = np.bincount(idx, minlength=B).astype(np.float64)
    seg_len = counts.astype(np.int64)

    n_windows = B // W                          # 128
    # Balance windows: any 32 segments may share a window (the host permutes
    # output rows back), so pack segments greedily (LPT) to minimize the
    # largest window's node count -> minimal t_win.
    import heapq

    order = np.argsort(-seg_len, kind="stable")
    heap = [(0, g) for g in range(n_windows)]
    heapq.heapify(heap)
    grp_of_seg = np.empty(B, np.int64)
    rel_of_seg = np.empty(B, np.int64)
    grp_sum = np.zeros(n_windows, np.int64)
    grp_cnt = np.zeros(n_windows, np.int64)
    for s in order:
        while True:
            tot, g = heapq.heappop(heap)
            if grp_cnt[g] < W and tot == grp_sum[g]:
                break
        grp_of_seg[s] = g
        rel_of_seg[s] = grp_cnt[g]
        grp_cnt[g] += 1
        grp_sum[g] += seg_len[s]
        if grp_cnt[g] < W:
            heapq.heappush(heap, (int(grp_sum[g]), g))

    t_win = int(np.ceil(grp_sum.max() / 128))
    tiles = WINDOWS_PER_CORE * t_win
    chunks = tiles // G

    # offset of each segment inside its window's node stream
    seg_off = np.zeros(B, np.int64)
    by_grp_rel = np.lexsort((rel_of_seg, grp_of_seg))
    lens_sorted = seg_len[by_grp_rel]
    csum = np.concatenate([[0], np.cumsum(lens_sorted)])[:-1]
    grp_base = csum[np.searchsorted(grp_of_seg[by_grp_rel], np.arange(n_windows))]
    seg_off[by_grp_rel] = csum - grp_base[grp_of_seg[by_grp_rel]]

    seg_start = np.concatenate([[0], np.cumsum(seg_len)])[:-1]
    g_of = grp_of_seg[idx]
    slot_in_win = seg_off[idx] + (np.arange(n) - seg_start[idx])

    # serpentine: slot s -> (p = s // t_win, t = s % t_win)
    p_of = slot_in_win // t_win
    t_of = slot_in_win % t_win
    core_of = g_of // WINDOWS_PER_CORE
    tt_of = (g_of % WINDOWS_PER_CORE) * t_win + t_of   # tile within core

    x_bf = x.astype(BF16)
    xarr = np.zeros((N_CORES, chunks, 128, G, C), dtype=BF16)
    xarr[core_of, tt_of // G, p_of, tt_of % G] = x_bf

    # per-slot one-hot from per-node rel scatter
    relarr = np.full((N_CORES, 128, tiles), -1, np.int32)
    relarr[core_of, p_of, tt_of] = rel_of_seg[idx].astype(np.int32)
    oh = (relarr[:, :, :, None] == np.arange(W)[None, None, None, :]).astype(BF16)

    xoh = np.empty((N_CORES, chunks, 128, OH + GC), dtype=BF16)
    xoh[:, :, :, OH:] = xarr.reshape(N_CORES, chunks, 128, GC)
    xoh[:, :, :, :OH] = (
        oh.reshape(N_CORES, 128, chunks, G, W)
        .transpose(0, 2, 1, 3, 4)
        .reshape(N_CORES, chunks, 128, OH)
    )
    # device row of segment s = grp_of_seg[s]*W + rel_of_seg[s]
    row_of_seg = grp_of_seg * W + rel_of_seg
    return t_win, xoh, counts, row_of_seg


def _finalize(sums, counts):
    """sums: [B, 4C] raw power sums (S1|S2|S3|S4) -> [B, 4C] moments f32."""
    s = sums.astype(np.float64)
    ncnt = np.maximum(counts, 1.0)[:, None]
    M1 = s[:, 0:C] / ncnt
    M2 = s[:, C : 2 * C] / ncnt
    M3 = s[:, 2 * C : 3 * C] / ncnt
    M4 = s[:, 3 * C : 4 * C] / ncnt
    mean = M1
    var = M2 - M1 * M1
    skew = M3 - 3.0 * M1 * M2 + 2.0 * M1 * M1 * M1
    kurt = (
        M4
        - 4.0 * M1 * M3
        + 6.0 * M1 * M1 * M2
        - 3.0 * M1 * M1 * M1 * M1
        - 3.0
    )
    return np.concatenate([mean, var, skew, kurt], axis=1).astype(np.float32)


def kernel(graph, batch_indices):
    from concourse.bass_utils import run_bass_kernel_spmd

    t_win, xoh, counts, row_of_seg = _prepare_inputs(graph, batch_indices)
    if t_win not in _prog_cache:
        _prog_cache[t_win] = _build_program(t_win)
    nc = _prog_cache[t_win]
    in_maps = [{"xoh": xoh[c]} for c in range(N_CORES)]
    res = run_bass_kernel_spmd(
        nc, in_maps, core_ids=list(range(N_CORES)), trace=TRACE
    )
    if TRACE:
        print(f"HW exec time: {res.exec_time_ns} ns")
    sums = np.concatenate([res.results[c]["out"] for c in range(N_CORES)], axis=0)
    return _finalize(sums[row_of_seg], counts)


# revision 24
# speedup vs baseline: 1.0247x; 1.0247x over previous
"""Trainium2 Bass kernel: segmented statistical moments (mean/var/skew/kurt).

Strategy (8 NeuronCores, one SPMD program):
  - 4096 sorted segments -> 512 consecutive segments per core, grouped into
    16 windows of W=32 segments. Host re-packs nodes serpentine-style:
    within a window, slot (p, t) holds node p*t_win + t of the window's
    segment stream; the per-slot one-hot (vs the window's 32 segments) is
    precomputed on the host in bf16 and shipped with x in one DMA per chunk.
  - All node data flows in bf16. Per chunk (G=16 tiles), ACT computes x^2
    (Square) plus a small slice of x^4; DVE computes x^3 = x*x2 and the
    rest of x^4 = x2*x2 (2x-packed bf16). GpSimd is left idle on purpose:
    its SBUF traffic poisons concurrent DVE tensor_tensor throughput.
  - Per 128-node tile: one matmul onehot.T @ [x | x^2 | x^3 | x^4] (bf16,
    free=256, 1 cycle/row; the per-matmul LDWEIGHTS ~96ns hides under the
    ~107ns compute) accumulates per-segment power sums in f32 PSUM.
    start=True on each window's first tile resets that window's 256-col
    half-bank (PSUM reset granularity is 256 columns; never interleave two
    accumulation chains inside one 256-col block).
  - Finished windows are copied PSUM->SBUF on ACT (keeps DVE clean) and
    DMA'd out. Host finalizes moments (float64) and concatenates cores.
"""

import sys

if "/opt/trn_rl_repo" not in sys.path:
    sys.path.insert(0, "/opt/trn_rl_repo")

import numpy as np
import ml_dtypes

BF16 = ml_dtypes.bfloat16

N_CORES = 8
B = 4096
C = 64
SEGS_PER_CORE = B // N_CORES      # 512
W = 32                            # segments per window
WINDOWS_PER_CORE = SEGS_PER_CORE // W   # 16
G = 16                            # 128-node tiles per chunk
GC = G * C                        # 1024
OH = G * W                        # 512 one-hot elems per partition per chunk
OH_PAD = GC                       # one-hot region padded: oh at [OH:2*OH)
ACT_X4_G = 3                      # g-slices of x^4 on ACT (rest on DVE)

_prog_cache = {}
TRACE = False


def _postprocess(nc, mybir, max_waits=1):
    """Walrus allows only one sync-wait per instruction; move extras onto
    standalone EventSemaphore instructions. Also flag one-hot matmul
    weights as {0,1}."""
    n = [0]

    def mk(engine, waits):
        wi = mybir.InstEventSemaphore(name=f"xw_{n[0]}", ins=[], outs=[])
        n[0] += 1
        wi.engine = engine
        wi.sync_info = mybir.SyncInfo(on_wait=list(waits), on_update=[])
        return wi

    for bb in nc.main_func.blocks:
        out = []
        for ins in bb.instructions:
            if ins.opcode == "Matmult":
                ins.is_weight_onezero = True
            si = ins.sync_info
            if (
                si is not None
                and len(si.on_wait) > max_waits
                and ins.opcode != "EventSemaphore"
            ):
                waits = list(si.on_wait)
                for w in waits[:-max_waits]:
                    out.append(mk(ins.engine, [w]))
                ins.sync_info = mybir.SyncInfo(
                    on_wait=waits[-max_waits:], on_update=list(si.on_update)
                )
            out.append(ins)
        bb.instructions = out


def _build_program(t_win):
    import concourse.bass as bass
    import concourse.tile as tile
    import concourse.mybir as mybir

    F32 = mybir.dt.float32
    BF = mybir.dt.bfloat16
    TT = mybir.AluOpType

    tiles = WINDOWS_PER_CORE * t_win
    assert tiles % G == 0
    chunks = tiles // G
    n_banks = WINDOWS_PER_CORE // 2   # two windows per PSUM bank

    nc = bass.Bass()
    # per chunk row: [oh (512)| x (1024)] bf16, landing at cmb[:, OH:OH+1536)
    xoh_d = nc.dram_tensor(
        "xoh", [chunks, 128, OH + GC], BF, kind="ExternalInput"
    )
    out_d = nc.dram_tensor("out", [SEGS_PER_CORE, 4 * C], F32, kind="ExternalOutput")

    with tile.TileContext(nc) as tc:
        with (
            tc.tile_pool(name="const", bufs=1) as const,
            tc.tile_pool(name="cmbp", bufs=12) as cmbp,
            tc.tile_pool(name="psp", bufs=1, space="PSUM") as psp,
            tc.tile_pool(name="outp", bufs=4) as outp,
        ):
            banks = [
                psp.tile([128, 512], F32, name=f"bank{k}", tag=f"bank{k}")
                for k in range(n_banks)
            ]

            # PE p-state warmup: ~50 dependency-free matmuls on zeroed SBUF
            # ramp the tensor engine to 2.4 GHz during the pipeline-fill dead
            # time. Results land in bank 0, which the first real window
            # matmul resets via start=True.
            warm = const.tile([128, 256], BF)
            nc.vector.memset(warm[:].bitcast(mybir.dt.uint16), 0)
            for i in range(12):
                nc.tensor.matmul(
                    banks[0][0:W, 0:256],
                    warm[:, 0:W],
                    warm[:, :],
                    start=(i == 0),
                    stop=(i == 11),
                    skip_group_check=True,
                )

            for k in range(chunks):
                cmb = cmbp.tile([128, OH_PAD + 4 * GC], BF)
                nc.sync.dma_start(
                    out=cmb[:, OH : OH + OH + GC], in_=xoh_d[k]
                )
                oh_t = cmb[:, OH : 2 * OH].rearrange("p (g w) -> p g w", w=W)
                o = OH_PAD
                xr = cmb[:, o : o + GC]
                x2r = cmb[:, o + GC : o + 2 * GC]
                x3r = cmb[:, o + 2 * GC : o + 3 * GC]
                x4r = cmb[:, o + 3 * GC : o + 4 * GC]

                nc.scalar.activation(
                    out=x2r, in_=xr, func=mybir.ActivationFunctionType.Square
                )
                nc.vector.tensor_tensor(out=x3r, in0=xr, in1=x2r, op=TT.mult)
                sa = ACT_X4_G * C
                if sa:
                    nc.scalar.activation(
                        out=x4r[:, 0:sa],
                        in_=x2r[:, 0:sa],
                        func=mybir.ActivationFunctionType.Square,
                    )
                nc.vector.tensor_tensor(
                    out=x4r[:, sa:], in0=x2r[:, sa:], in1=x2r[:, sa:], op=TT.mult
                )

                pow4 = cmb[:, o : o + 4 * GC].rearrange("p (s gc) -> p s gc", s=4)
                for g in range(G):
                    t = k * G + g
                    w = t // t_win
                    bank = banks[w // 2]
                    col0 = (w % 2) * 256
                    nc.tensor.matmul(
                        bank[0:W, col0 : col0 + 256],
                        oh_t[:, g, :],
                        pow4[:, :, g * C : (g + 1) * C],
                        start=t % t_win == 0,
                        stop=(t + 1) % t_win == 0,
                        skip_group_check=True,
                    )
                    # copy a bank only after BOTH its windows are done, so no
                    # copy ever false-shares the bank tile with a start-matmul
                    if (t + 1) % (2 * t_win) == 0:
                        kb = w // 2
                        o_t = outp.tile([W, 512], F32, name=f"o{kb % 4}", tag="o")
                        nc.scalar.copy(o_t[:, :], bank[0:W, :])
                        row0 = kb * 2 * W
                        od = out_d[:]
                        out_ap = bass.AP(
                            tensor=od.tensor,
                            offset=od.offset + row0 * 256,
                            ap=[[256, W], [W * 256, 2], [1, 256]],
                        )
                        in_ap = o_t[:].rearrange("p (j c) -> p j c", j=2)
                        nc.gpsimd.dma_start(out=out_ap, in_=in_ap)

    _postprocess(nc, mybir)
    return nc


def _prepare_inputs(graph, batch_indices):
    idx = np.asarray(batch_indices).astype(np.int64)
    x = np.ascontiguousarray(np.asarray(graph, dtype=np.float32))
    n = idx.shape[0]

    counts = np.bincount(idx, minlength=B).astype(np.float64)
    seg_len = counts.astype(np.int64)

    n_windows = B // W                          # 128
    # Balance windows: any 32 segments may share a window (the host permutes
    # output rows back), so pack segments greedily (LPT) to minimize the
    # largest window's node count -> minimal t_win.
    import heapq

    order = np.argsort(-seg_len, kind="stable")
    heap = [(0, g) for g in range(n_windows)]
    heapq.heapify(heap)
    grp_of_seg = np.empty(B, np.int64)
    rel_of_seg = np.empty(B, np.int64)
    grp_sum = np.zeros(n_windows, np.int64)
    grp_cnt = np.zeros(n_windows, np.int64)
    for s in order:
        while True:
            tot, g = heapq.heappop(heap)
            if grp_cnt[g] < W and tot == grp_sum[g]:
                break
        grp_of_seg[s] = g
        rel_of_seg[s] = grp_cnt[g]
        grp_cnt[g] += 1
        grp_sum[g] += seg_len[s]
        if grp_cnt[g] < W:
            heapq.heappush(heap, (int(grp_sum[g]), g))

    t_win = int(np.ceil(grp_sum.max() / 128))
    tiles = WINDOWS_PER_CORE * t_win
    chunks = tiles // G

    # offset of each segment inside its window's node stream
    seg_off = np.zeros(B, np.int64)
    by_grp_rel = np.lexsort((rel_of_seg, grp_of_seg))
    lens_sorted = seg_len[by_grp_rel]
    csum = np.concatenate([[0], np.cumsum(lens_sorted)])[:-1]
    grp_base = csum[np.searchsorted(grp_of_seg[by_grp_rel], np.arange(n_windows))]
    seg_off[by_grp_rel] = csum - grp_base[grp_of_seg[by_grp_rel]]

    seg_start = np.concatenate([[0], np.cumsum(seg_len)])[:-1]
    g_of = grp_of_seg[idx]
    slot_in_win = seg_off[idx] + (np.arange(n) - seg_start[idx])

    # serpentine: slot s -> (p = s // t_win, t = s % t_win)
    p_of = slot_in_win // t_win
    t_of = slot_in_win % t_win
    core_of = g_of // WINDOWS_PER_CORE
    tt_of = (g_of % WINDOWS_PER_CORE) * t_win + t_of   # tile within core

    x_bf = x.astype(BF16)
    xarr = np.zeros((N_CORES, chunks, 128, G, C), dtype=BF16)
    xarr[core_of, tt_of // G, p_of, tt_of % G] = x_bf

    # per-slot one-hot from per-node rel scatter
    relarr = np.full((N_CORES, 128, tiles), -1, np.int32)
    relarr[core_of, p_of, tt_of] = rel_of_seg[idx].astype(np.int32)
    oh = (relarr[:, :, :, None] == np.arange(W)[None, None, None, :]).astype(BF16)

    xoh = np.empty((N_CORES, chunks, 128, OH + GC), dtype=BF16)
    xoh[:, :, :, OH:] = xarr.reshape(N_CORES, chunks, 128, GC)
    xoh[:, :, :, :OH] = (
        oh.reshape(N_CORES, 128, chunks, G, W)
        .transpose(0, 2, 1, 3, 4)
        .reshape(N_CORES, chunks, 128, OH)
    )
    # device row of segment s = grp_of_seg[s]*W + rel_of_seg[s]
    row_of_seg = grp_of_seg * W + rel_of_seg
    return t_win, xoh, counts, row_of_seg


def _finalize(sums, counts):
    """sums: [B, 4C] raw power sums (S1|S2|S3|S4) -> [B, 4C] moments f32."""
    s = sums.astype(np.float64)
    ncnt = np.maximum(counts, 1.0)[:, None]
    M1 = s[:, 0:C] / ncnt
    M2 = s[:, C : 2 * C] / ncnt
    M3 = s[:, 2 * C : 3 * C] / ncnt
    M4 = s[:, 3 * C : 4 * C] / ncnt
    mean = M1
    var = M2 - M1 * M1
    skew = M3 - 3.0 * M1 * M2 + 2.0 * M1 * M1 * M1
    kurt = (
        M4
        - 4.0 * M1 * M3
        + 6.0 * M1 * M1 * M2
        - 3.0 * M1 * M1 * M1 * M1
        - 3.0
    )
    return np.concatenate([mean, var, skew, kurt], axis=1).astype(np.float32)


def kernel(graph, batch_indices):
    from concourse.bass_utils import run_bass_kernel_spmd

    t_win, xoh, counts, row_of_seg = _prepare_inputs(graph, batch_indices)
    if t_win not in _prog_cache:
        _prog_cache[t_win] = _build_program(t_win)
    nc = _prog_cache[t_win]
    in_maps = [{"xoh": xoh[c]} for c in range(N_CORES)]
    res = run_bass_kernel_spmd(
        nc, in_maps, core_ids=list(range(N_CORES)), trace=TRACE
    )
    if TRACE:
        print(f"HW exec time: {res.exec_time_ns} ns")
    sums = np.concatenate([res.results[c]["out"] for c in range(N_CORES)], axis=0)
    return _finalize(sums[row_of_seg], counts)


# revision 25
# speedup vs baseline: 1.0411x; 1.0160x over previous
"""Trainium2 Bass kernel: segmented statistical moments (mean/var/skew/kurt).

Strategy (8 NeuronCores, one SPMD program):
  - 4096 sorted segments -> 512 consecutive segments per core, grouped into
    16 windows of W=32 segments. Host re-packs nodes serpentine-style:
    within a window, slot (p, t) holds node p*t_win + t of the window's
    segment stream; the per-slot one-hot (vs the window's 32 segments) is
    precomputed on the host in bf16 and shipped with x in one DMA per chunk.
  - All node data flows in bf16. Per chunk (G=16 tiles), ACT computes x^2
    (Square) plus a small slice of x^4; DVE computes x^3 = x*x2 and the
    rest of x^4 = x2*x2 (2x-packed bf16). GpSimd is left idle on purpose:
    its SBUF traffic poisons concurrent DVE tensor_tensor throughput.
  - Per 128-node tile: one matmul onehot.T @ [x | x^2 | x^3 | x^4] (bf16,
    free=256, 1 cycle/row; the per-matmul LDWEIGHTS ~96ns hides under the
    ~107ns compute) accumulates per-segment power sums in f32 PSUM.
    start=True on each window's first tile resets that window's 256-col
    half-bank (PSUM reset granularity is 256 columns; never interleave two
    accumulation chains inside one 256-col block).
  - Finished windows are copied PSUM->SBUF on ACT (keeps DVE clean) and
    DMA'd out. Host finalizes moments (float64) and concatenates cores.
"""

import sys

if "/opt/trn_rl_repo" not in sys.path:
    sys.path.insert(0, "/opt/trn_rl_repo")

import numpy as np
import ml_dtypes

BF16 = ml_dtypes.bfloat16

N_CORES = 8
B = 4096
C = 64
SEGS_PER_CORE = B // N_CORES      # 512
W = 32                            # segments per window
WINDOWS_PER_CORE = SEGS_PER_CORE // W   # 16
G = 16                            # 128-node tiles per chunk
GC = G * C                        # 1024
OH = G * W                        # 512 one-hot elems per partition per chunk
OH_PAD = GC                       # one-hot region padded: oh at [OH:2*OH)
ACT_X4_G = 0                      # g-slices of x^4 on ACT (rest on DVE)

_prog_cache = {}
TRACE = False


def _postprocess(nc, mybir, max_waits=1):
    """Walrus allows only one sync-wait per instruction; move extras onto
    standalone EventSemaphore instructions. Also flag one-hot matmul
    weights as {0,1}."""
    n = [0]

    def mk(engine, waits):
        wi = mybir.InstEventSemaphore(name=f"xw_{n[0]}", ins=[], outs=[])
        n[0] += 1
        wi.engine = engine
        wi.sync_info = mybir.SyncInfo(on_wait=list(waits), on_update=[])
        return wi

    for bb in nc.main_func.blocks:
        out = []
        for ins in bb.instructions:
            if ins.opcode == "Matmult":
                ins.is_weight_onezero = True
            si = ins.sync_info
            if (
                si is not None
                and len(si.on_wait) > max_waits
                and ins.opcode != "EventSemaphore"
            ):
                waits = list(si.on_wait)
                for w in waits[:-max_waits]:
                    out.append(mk(ins.engine, [w]))
                ins.sync_info = mybir.SyncInfo(
                    on_wait=waits[-max_waits:], on_update=list(si.on_update)
                )
            out.append(ins)
        bb.instructions = out


def _build_program(t_win):
    import concourse.bass as bass
    import concourse.tile as tile
    import concourse.mybir as mybir

    F32 = mybir.dt.float32
    BF = mybir.dt.bfloat16
    TT = mybir.AluOpType

    tiles = WINDOWS_PER_CORE * t_win
    assert tiles % G == 0
    chunks = tiles // G
    n_banks = WINDOWS_PER_CORE // 2   # two windows per PSUM bank

    nc = bass.Bass()
    # per chunk row: [oh (512)| x (1024)] bf16, landing at cmb[:, OH:OH+1536)
    xoh_d = nc.dram_tensor(
        "xoh", [chunks, 128, OH + GC], BF, kind="ExternalInput"
    )
    out_d = nc.dram_tensor("out", [SEGS_PER_CORE, 4 * C], F32, kind="ExternalOutput")

    with tile.TileContext(nc) as tc:
        with (
            tc.tile_pool(name="const", bufs=1) as const,
            tc.tile_pool(name="cmbp", bufs=12) as cmbp,
            tc.tile_pool(name="psp", bufs=1, space="PSUM") as psp,
            tc.tile_pool(name="outp", bufs=4) as outp,
        ):
            banks = [
                psp.tile([128, 512], F32, name=f"bank{k}", tag=f"bank{k}")
                for k in range(n_banks)
            ]

            # PE p-state warmup: ~50 dependency-free matmuls on zeroed SBUF
            # ramp the tensor engine to 2.4 GHz during the pipeline-fill dead
            # time. Results land in bank 0, which the first real window
            # matmul resets via start=True.
            warm = const.tile([128, 256], BF)
            nc.vector.memset(warm[:].bitcast(mybir.dt.uint16), 0)
            for i in range(12):
                nc.tensor.matmul(
                    banks[0][0:W, 0:256],
                    warm[:, 0:W],
                    warm[:, :],
                    start=(i == 0),
                    stop=(i == 11),
                    skip_group_check=True,
                )

            for k in range(chunks):
                cmb = cmbp.tile([128, OH_PAD + 4 * GC], BF)
                nc.sync.dma_start(
                    out=cmb[:, OH : OH + OH + GC], in_=xoh_d[k]
                )
                oh_t = cmb[:, OH : 2 * OH].rearrange("p (g w) -> p g w", w=W)
                o = OH_PAD
                xr = cmb[:, o : o + GC]
                x2r = cmb[:, o + GC : o + 2 * GC]
                x3r = cmb[:, o + 2 * GC : o + 3 * GC]
                x4r = cmb[:, o + 3 * GC : o + 4 * GC]

                nc.scalar.activation(
                    out=x2r, in_=xr, func=mybir.ActivationFunctionType.Square
                )
                nc.vector.tensor_tensor(out=x3r, in0=xr, in1=x2r, op=TT.mult)
                sa = ACT_X4_G * C
                if sa:
                    nc.scalar.activation(
                        out=x4r[:, 0:sa],
                        in_=x2r[:, 0:sa],
                        func=mybir.ActivationFunctionType.Square,
                    )
                nc.vector.tensor_tensor(
                    out=x4r[:, sa:], in0=x2r[:, sa:], in1=x2r[:, sa:], op=TT.mult
                )

                pow4 = cmb[:, o : o + 4 * GC].rearrange("p (s gc) -> p s gc", s=4)
                for g in range(G):
                    t = k * G + g
                    w = t // t_win
                    bank = banks[w // 2]
                    col0 = (w % 2) * 256
                    nc.tensor.matmul(
                        bank[0:W, col0 : col0 + 256],
                        oh_t[:, g, :],
                        pow4[:, :, g * C : (g + 1) * C],
                        start=t % t_win == 0,
                        stop=(t + 1) % t_win == 0,
                        skip_group_check=True,
                    )
                    # copy a bank only after BOTH its windows are done, so no
                    # copy ever false-shares the bank tile with a start-matmul
                    if (t + 1) % (2 * t_win) == 0:
                        kb = w // 2
                        o_t = outp.tile([W, 512], F32, name=f"o{kb % 4}", tag="o")
                        nc.scalar.copy(o_t[:, :], bank[0:W, :])
                        row0 = kb * 2 * W
                        od = out_d[:]
                        out_ap = bass.AP(
                            tensor=od.tensor,
                            offset=od.offset + row0 * 256,
                            ap=[[256, W], [W * 256, 2], [1, 256]],
                        )
                        in_ap = o_t[:].rearrange("p (j c) -> p j c", j=2)
                        nc.gpsimd.dma_start(out=out_ap, in_=in_ap)

    _postprocess(nc, mybir)
    return nc


def _prepare_inputs(graph, batch_indices):
    idx = np.asarray(batch_indices).astype(np.int64)
    x = np.ascontiguousarray(np.asarray(graph, dtype=np.float32))
    n = idx.shape[0]

    counts = np.bincount(idx, minlength=B).astype(np.float64)
    seg_len = counts.astype(np.int64)

    n_windows = B // W                          # 128
    # Balance windows: any 32 segments may share a window (the host permutes
    # output rows back), so pack segments greedily (LPT) to minimize the
    # largest window's node count -> minimal t_win.
    import heapq

    order = np.argsort(-seg_len, kind="stable")
    heap = [(0, g) for g in range(n_windows)]
    heapq.heapify(heap)
    grp_of_seg = np.empty(B, np.int64)
    rel_of_seg = np.empty(B, np.int64)
    grp_sum = np.zeros(n_windows, np.int64)
    grp_cnt = np.zeros(n_windows, np.int64)
    for s in order:
        while True:
            tot, g = heapq.heappop(heap)
            if grp_cnt[g] < W and tot == grp_sum[g]:
                break
        grp_of_seg[s] = g
        rel_of_seg[s] = grp_cnt[g]
        grp_cnt[g] += 1
        grp_sum[g] += seg_len[s]
        if grp_cnt[g] < W:
            heapq.heappush(heap, (int(grp_sum[g]), g))

    t_win = int(np.ceil(grp_sum.max() / 128))
    tiles = WINDOWS_PER_CORE * t_win
    chunks = tiles // G

    # offset of each segment inside its window's node stream
    seg_off = np.zeros(B, np.int64)
    by_grp_rel = np.lexsort((rel_of_seg, grp_of_seg))
    lens_sorted = seg_len[by_grp_rel]
    csum = np.concatenate([[0], np.cumsum(lens_sorted)])[:-1]
    grp_base = csum[np.searchsorted(grp_of_seg[by_grp_rel], np.arange(n_windows))]
    seg_off[by_grp_rel] = csum - grp_base[grp_of_seg[by_grp_rel]]

    seg_start = np.concatenate([[0], np.cumsum(seg_len)])[:-1]
    g_of = grp_of_seg[idx]
    slot_in_win = seg_off[idx] + (np.arange(n) - seg_start[idx])

    # serpentine: slot s -> (p = s // t_win, t = s % t_win)
    p_of = slot_in_win // t_win
    t_of = slot_in_win % t_win
    core_of = g_of // WINDOWS_PER_CORE
    tt_of = (g_of % WINDOWS_PER_CORE) * t_win + t_of   # tile within core

    x_bf = x.astype(BF16)
    xarr = np.zeros((N_CORES, chunks, 128, G, C), dtype=BF16)
    xarr[core_of, tt_of // G, p_of, tt_of % G] = x_bf

    # per-slot one-hot from per-node rel scatter
    relarr = np.full((N_CORES, 128, tiles), -1, np.int32)
    relarr[core_of, p_of, tt_of] = rel_of_seg[idx].astype(np.int32)
    oh = (relarr[:, :, :, None] == np.arange(W)[None, None, None, :]).astype(BF16)

    xoh = np.empty((N_CORES, chunks, 128, OH + GC), dtype=BF16)
    xoh[:, :, :, OH:] = xarr.reshape(N_CORES, chunks, 128, GC)
    xoh[:, :, :, :OH] = (
        oh.reshape(N_CORES, 128, chunks, G, W)
        .transpose(0, 2, 1, 3, 4)
        .reshape(N_CORES, chunks, 128, OH)
    )
    # device row of segment s = grp_of_seg[s]*W + rel_of_seg[s]
    row_of_seg = grp_of_seg * W + rel_of_seg
    return t_win, xoh, counts, row_of_seg


def _finalize(sums, counts):
    """sums: [B, 4C] raw power sums (S1|S2|S3|S4) -> [B, 4C] moments f32."""
    s = sums.astype(np.float64)
    ncnt = np.maximum(counts, 1.0)[:, None]
    M1 = s[:, 0:C] / ncnt
    M2 = s[:, C : 2 * C] / ncnt
    M3 = s[:, 2 * C : 3 * C] / ncnt
    M4 = s[:, 3 * C : 4 * C] / ncnt
    mean = M1
    var = M2 - M1 * M1
    skew = M3 - 3.0 * M1 * M2 + 2.0 * M1 * M1 * M1
    kurt = (
        M4
        - 4.0 * M1 * M3
        + 6.0 * M1 * M1 * M2
        - 3.0 * M1 * M1 * M1 * M1
        - 3.0
    )
    return np.concatenate([mean, var, skew, kurt], axis=1).astype(np.float32)


def kernel(graph, batch_indices):
    from concourse.bass_utils import run_bass_kernel_spmd

    t_win, xoh, counts, row_of_seg = _prepare_inputs(graph, batch_indices)
    if t_win not in _prog_cache:
        _prog_cache[t_win] = _build_program(t_win)
    nc = _prog_cache[t_win]
    in_maps = [{"xoh": xoh[c]} for c in range(N_CORES)]
    res = run_bass_kernel_spmd(
        nc, in_maps, core_ids=list(range(N_CORES)), trace=TRACE
    )
    if TRACE:
        print(f"HW exec time: {res.exec_time_ns} ns")
    sums = np.concatenate([res.results[c]["out"] for c in range(N_CORES)], axis=0)
    return _finalize(sums[row_of_seg], counts)


# revision 26
# speedup vs baseline: 1.0618x; 1.0198x over previous
"""Trainium2 Bass kernel: segmented statistical moments (mean/var/skew/kurt).

Strategy (8 NeuronCores, one SPMD program):
  - 4096 sorted segments -> 512 consecutive segments per core, grouped into
    16 windows of W=32 segments. Host re-packs nodes serpentine-style:
    within a window, slot (p, t) holds node p*t_win + t of the window's
    segment stream; the per-slot one-hot (vs the window's 32 segments) is
    precomputed on the host in bf16 and shipped with x in one DMA per chunk.
  - All node data flows in bf16. Per chunk (G=16 tiles), ACT computes x^2
    (Square) plus a small slice of x^4; DVE computes x^3 = x*x2 and the
    rest of x^4 = x2*x2 (2x-packed bf16). GpSimd is left idle on purpose:
    its SBUF traffic poisons concurrent DVE tensor_tensor throughput.
  - Per 128-node tile: one matmul onehot.T @ [x | x^2 | x^3 | x^4] (bf16,
    free=256, 1 cycle/row; the per-matmul LDWEIGHTS ~96ns hides under the
    ~107ns compute) accumulates per-segment power sums in f32 PSUM.
    start=True on each window's first tile resets that window's 256-col
    half-bank (PSUM reset granularity is 256 columns; never interleave two
    accumulation chains inside one 256-col block).
  - Finished windows are copied PSUM->SBUF on ACT (keeps DVE clean) and
    DMA'd out. Host finalizes moments (float64) and concatenates cores.
"""

import sys

if "/opt/trn_rl_repo" not in sys.path:
    sys.path.insert(0, "/opt/trn_rl_repo")

import numpy as np
import ml_dtypes

BF16 = ml_dtypes.bfloat16

N_CORES = 8
B = 4096
C = 64
SEGS_PER_CORE = B // N_CORES      # 512
W = 32                            # segments per window
WINDOWS_PER_CORE = SEGS_PER_CORE // W   # 16
G = 16                            # 128-node tiles per chunk
GC = G * C                        # 1024
OH = G * W                        # 512 one-hot elems per partition per chunk
OH_PAD = GC                       # one-hot region padded: oh at [OH:2*OH)
ACT_X4_G = 0                      # g-slices of x^4 on ACT (rest on DVE)

_prog_cache = {}
TRACE = False


def _postprocess(nc, mybir, max_waits=1):
    """Walrus allows only one sync-wait per instruction; move extras onto
    standalone EventSemaphore instructions. Also flag one-hot matmul
    weights as {0,1}."""
    n = [0]

    def mk(engine, waits):
        wi = mybir.InstEventSemaphore(name=f"xw_{n[0]}", ins=[], outs=[])
        n[0] += 1
        wi.engine = engine
        wi.sync_info = mybir.SyncInfo(on_wait=list(waits), on_update=[])
        return wi

    for bb in nc.main_func.blocks:
        out = []
        for ins in bb.instructions:
            if ins.opcode == "Matmult":
                ins.is_weight_onezero = True
            si = ins.sync_info
            if (
                si is not None
                and len(si.on_wait) > max_waits
                and ins.opcode != "EventSemaphore"
            ):
                waits = list(si.on_wait)
                for w in waits[:-max_waits]:
                    out.append(mk(ins.engine, [w]))
                ins.sync_info = mybir.SyncInfo(
                    on_wait=waits[-max_waits:], on_update=list(si.on_update)
                )
            out.append(ins)
        bb.instructions = out


def _build_program(t_win):
    import concourse.bass as bass
    import concourse.tile as tile
    import concourse.mybir as mybir

    F32 = mybir.dt.float32
    BF = mybir.dt.bfloat16
    TT = mybir.AluOpType

    tiles = WINDOWS_PER_CORE * t_win
    assert tiles % G == 0
    chunks = tiles // G
    n_banks = WINDOWS_PER_CORE // 2   # two windows per PSUM bank

    nc = bass.Bass()
    # per chunk row: [oh (512)| x (1024)] bf16, landing at cmb[:, OH:OH+1536)
    xoh_d = nc.dram_tensor(
        "xoh", [chunks, 128, OH + GC], BF, kind="ExternalInput"
    )
    out_d = nc.dram_tensor("out", [SEGS_PER_CORE, 4 * C], F32, kind="ExternalOutput")

    with tile.TileContext(nc) as tc:
        with (
            tc.tile_pool(name="const", bufs=1) as const,
            tc.tile_pool(name="cmbp", bufs=12) as cmbp,
            tc.tile_pool(name="psp", bufs=1, space="PSUM") as psp,
            tc.tile_pool(name="outp", bufs=4) as outp,
        ):
            banks = [
                psp.tile([128, 512], F32, name=f"bank{k}", tag=f"bank{k}")
                for k in range(n_banks)
            ]

            # PE p-state warmup: ~50 dependency-free matmuls on zeroed SBUF
            # ramp the tensor engine to 2.4 GHz during the pipeline-fill dead
            # time. Results land in bank 0, which the first real window
            # matmul resets via start=True.
            warm = const.tile([128, 256], BF)
            nc.vector.memset(warm[:].bitcast(mybir.dt.uint16), 0)
            for i in range(16):
                nc.tensor.matmul(
                    banks[0][0:W, 0:256],
                    warm[:, 0:W],
                    warm[:, :],
                    start=(i == 0),
                    stop=(i == 15),
                    skip_group_check=True,
                )

            for k in range(chunks):
                cmb = cmbp.tile([128, OH_PAD + 4 * GC], BF)
                nc.sync.dma_start(
                    out=cmb[:, OH : OH + OH + GC], in_=xoh_d[k]
                )
                oh_t = cmb[:, OH : 2 * OH].rearrange("p (g w) -> p g w", w=W)
                o = OH_PAD
                xr = cmb[:, o : o + GC]
                x2r = cmb[:, o + GC : o + 2 * GC]
                x3r = cmb[:, o + 2 * GC : o + 3 * GC]
                x4r = cmb[:, o + 3 * GC : o + 4 * GC]

                nparts = 4 if k == 0 else 1
                step = GC // nparts
                for q in range(nparts):
                    lo, hi = q * step, (q + 1) * step
                    nc.scalar.activation(
                        out=x2r[:, lo:hi],
                        in_=xr[:, lo:hi],
                        func=mybir.ActivationFunctionType.Square,
                    )
                    nc.vector.tensor_tensor(
                        out=x3r[:, lo:hi], in0=xr[:, lo:hi], in1=x2r[:, lo:hi],
                        op=TT.mult,
                    )
                    nc.vector.tensor_tensor(
                        out=x4r[:, lo:hi], in0=x2r[:, lo:hi], in1=x2r[:, lo:hi],
                        op=TT.mult,
                    )

                pow4 = cmb[:, o : o + 4 * GC].rearrange("p (s gc) -> p s gc", s=4)
                for g in range(G):
                    t = k * G + g
                    w = t // t_win
                    bank = banks[w // 2]
                    col0 = (w % 2) * 256
                    nc.tensor.matmul(
                        bank[0:W, col0 : col0 + 256],
                        oh_t[:, g, :],
                        pow4[:, :, g * C : (g + 1) * C],
                        start=t % t_win == 0,
                        stop=(t + 1) % t_win == 0,
                        skip_group_check=True,
                    )
                    # copy a bank only after BOTH its windows are done, so no
                    # copy ever false-shares the bank tile with a start-matmul
                    if (t + 1) % (2 * t_win) == 0:
                        kb = w // 2
                        o_t = outp.tile([W, 512], F32, name=f"o{kb % 4}", tag="o")
                        nc.scalar.copy(o_t[:, :], bank[0:W, :])
                        row0 = kb * 2 * W
                        od = out_d[:]
                        out_ap = bass.AP(
                            tensor=od.tensor,
                            offset=od.offset + row0 * 256,
                            ap=[[256, W], [W * 256, 2], [1, 256]],
                        )
                        in_ap = o_t[:].rearrange("p (j c) -> p j c", j=2)
                        nc.gpsimd.dma_start(out=out_ap, in_=in_ap)

    _postprocess(nc, mybir)
    return nc


def _prepare_inputs(graph, batch_indices):
    idx = np.asarray(batch_indices).astype(np.int64)
    x = np.ascontiguousarray(np.asarray(graph, dtype=np.float32))
    n = idx.shape[0]

    counts = np.bincount(idx, minlength=B).astype(np.float64)
    seg_len = counts.astype(np.int64)

    n_windows = B // W                          # 128
    # Balance windows: any 32 segments may share a window (the host permutes
    # output rows back), so pack segments greedily (LPT) to minimize the
    # largest window's node count -> minimal t_win.
    import heapq

    order = np.argsort(-seg_len, kind="stable")
    heap = [(0, g) for g in range(n_windows)]
    heapq.heapify(heap)
    grp_of_seg = np.empty(B, np.int64)
    rel_of_seg = np.empty(B, np.int64)
    grp_sum = np.zeros(n_windows, np.int64)
    grp_cnt = np.zeros(n_windows, np.int64)
    for s in order:
        while True:
            tot, g = heapq.heappop(heap)
            if grp_cnt[g] < W and tot == grp_sum[g]:
                break
        grp_of_seg[s] = g
        rel_of_seg[s] = grp_cnt[g]
        grp_cnt[g] += 1
        grp_sum[g] += seg_len[s]
        if grp_cnt[g] < W:
            heapq.heappush(heap, (int(grp_sum[g]), g))

    t_win = int(np.ceil(grp_sum.max() / 128))
    tiles = WINDOWS_PER_CORE * t_win
    chunks = tiles // G

    # offset of each segment inside its window's node stream
    seg_off = np.zeros(B, np.int64)
    by_grp_rel = np.lexsort((rel_of_seg, grp_of_seg))
    lens_sorted = seg_len[by_grp_rel]
    csum = np.concatenate([[0], np.cumsum(lens_sorted)])[:-1]
    grp_base = csum[np.searchsorted(grp_of_seg[by_grp_rel], np.arange(n_windows))]
    seg_off[by_grp_rel] = csum - grp_base[grp_of_seg[by_grp_rel]]

    seg_start = np.concatenate([[0], np.cumsum(seg_len)])[:-1]
    g_of = grp_of_seg[idx]
    slot_in_win = seg_off[idx] + (np.arange(n) - seg_start[idx])

    # serpentine: slot s -> (p = s // t_win, t = s % t_win)
    p_of = slot_in_win // t_win
    t_of = slot_in_win % t_win
    core_of = g_of // WINDOWS_PER_CORE
    tt_of = (g_of % WINDOWS_PER_CORE) * t_win + t_of   # tile within core

    x_bf = x.astype(BF16)
    xarr = np.zeros((N_CORES, chunks, 128, G, C), dtype=BF16)
    xarr[core_of, tt_of // G, p_of, tt_of % G] = x_bf

    # per-slot one-hot from per-node rel scatter
    relarr = np.full((N_CORES, 128, tiles), -1, np.int32)
    relarr[core_of, p_of, tt_of] = rel_of_seg[idx].astype(np.int32)
    oh = (relarr[:, :, :, None] == np.arange(W)[None, None, None, :]).astype(BF16)

    xoh = np.empty((N_CORES, chunks, 128, OH + GC), dtype=BF16)
    xoh[:, :, :, OH:] = xarr.reshape(N_CORES, chunks, 128, GC)
    xoh[:, :, :, :OH] = (
        oh.reshape(N_CORES, 128, chunks, G, W)
        .transpose(0, 2, 1, 3, 4)
        .reshape(N_CORES, chunks, 128, OH)
    )
    # device row of segment s = grp_of_seg[s]*W + rel_of_seg[s]
    row_of_seg = grp_of_seg * W + rel_of_seg
    return t_win, xoh, counts, row_of_seg


def _finalize(sums, counts):
    """sums: [B, 4C] raw power sums (S1|S2|S3|S4) -> [B, 4C] moments f32."""
    s = sums.astype(np.float64)
    ncnt = np.maximum(counts, 1.0)[:, None]
    M1 = s[:, 0:C] / ncnt
    M2 = s[:, C : 2 * C] / ncnt
    M3 = s[:, 2 * C : 3 * C] / ncnt
    M4 = s[:, 3 * C : 4 * C] / ncnt
    mean = M1
    var = M2 - M1 * M1
    skew = M3 - 3.0 * M1 * M2 + 2.0 * M1 * M1 * M1
    kurt = (
        M4
        - 4.0 * M1 * M3
        + 6.0 * M1 * M1 * M2
        - 3.0 * M1 * M1 * M1 * M1
        - 3.0
    )
    return np.concatenate([mean, var, skew, kurt], axis=1).astype(np.float32)


def kernel(graph, batch_indices):
    from concourse.bass_utils import run_bass_kernel_spmd

    t_win, xoh, counts, row_of_seg = _prepare_inputs(graph, batch_indices)
    if t_win not in _prog_cache:
        _prog_cache[t_win] = _build_program(t_win)
    nc = _prog_cache[t_win]
    in_maps = [{"xoh": xoh[c]} for c in range(N_CORES)]
    res = run_bass_kernel_spmd(
        nc, in_maps, core_ids=list(range(N_CORES)), trace=TRACE
    )
    if TRACE:
        print(f"HW exec time: {res.exec_time_ns} ns")
    sums = np.concatenate([res.results[c]["out"] for c in range(N_CORES)], axis=0)
    return _finalize(sums[row_of_seg], counts)
